# revision 11
# baseline (speedup 1.0000x reference)
"""Multi-head causal attention (B=4, T=2048, D=1024, H=16, HS=64) on 8 TRN2
NeuronCores.

Sharding: batch (4-way) x head-group (2-way).  Core c handles batch c//2 and
heads 8*(c%2) .. 8*(c%2)+7.  Each core computes its 8 heads' attention and the
partial output projection Y_T = sum_h Wo_h^T @ O_T_h; the host sums the two
head-group partials per batch, transposes, and adds the output bias.

Per-core program (matmul datapath bf16, fp32 PSUM accumulation):
  - x^T arrives pre-transposed from host as [d, t] chunks.
  - V is computed directly in [t(=k), e] layout via matmul(lhsT=x^T chunk,
    rhs=Wv[d, e8]) -- no PE transposes at all.  A ones column per head turns
    the attn@v matmul into a fused (O^T, l) computation.
  - Q^T/K^T [e2, t] = matmul(lhsT=W[d, e2], rhs=x^T), head pairs packed on
    the PE M axis (head h of the pair in partitions 64h..64h+63).
  - S^T blocks [k=128, q<=512]: the two heads of a pair run CONCURRENTLY in
    the PE array via row tiling -- head 0 in rows 0-63 (tile_position (0,0)),
    head 1 in rows 64-127 ((64,0)), separate PSUM banks.  K=64 therefore
    costs ~N cycles per head PAIR instead of per head.
  - exp on ScalarE covers both heads in one ACTIVATE ([128, 2, 512-off] AP,
    1/sqrt(HS) folded into the activation scale); causal mask = upper-tri
    0/1 multiply on the diagonal sub-block only.
  - O^T_aug [65, q] accumulates per head over k chunks in PSUM; softmax
    normalization: reciprocal_approx_fast of the l row + DRAM-bounced
    partition broadcast + one DVE multiply into otn (bf16).
  - Output projection: per (dc, qc) a single 4-matmul PSUM chain over all
    pairs, bf16 copy, DMA out.  Emitted as PE filler inside pair-3's
    attention stream (lagging one q-chunk) and drained at the end.

Engine-level scheduling: emission order is the schedule.  S^T runs two
chunks ahead of attn@v; independent PE work (next pair's Q/K projections,
output-projection chains) is emitted as filler inside the attention stream
so the PE never idles and the HAM clock gate stays at 2.4 GHz.
"""

import numpy as np

B, T, D = 4, 2048, 1024
H, HS = 16, 64
NCORES = 8
NPAIR = 4   # head pairs per core
ND = 8      # 128-wide d chunks
NQ = 4      # 512-wide q chunks
NK = 16     # 128-wide k chunks

_CACHE = {}


def _build_program():
    import concourse.bass as bass
    import concourse.tile as tile
    from concourse import bacc, mybir
    from contextlib import ExitStack

    f32 = mybir.dt.float32
    bf16 = mybir.dt.bfloat16
    Exp = mybir.ActivationFunctionType.Exp

    nc = bacc.Bacc("TRN2", target_bir_lowering=False, debug=False)

    x_d = nc.declare_dram_parameter("x", [128, NQ, ND, 512], bf16, isOutput=False)
    wq_d = nc.declare_dram_parameter("wq", [NPAIR, 128, ND, 128], bf16, isOutput=False)
    wk_d = nc.declare_dram_parameter("wk", [NPAIR, 128, ND, 128], bf16, isOutput=False)
    wv_d = nc.declare_dram_parameter("wv", [128, ND, 512], bf16, isOutput=False)
    wo_d = nc.declare_dram_parameter("wo", [128, NPAIR, ND, 128], bf16, isOutput=False)
    tri_d = nc.declare_dram_parameter("tri", [128, 128], bf16, isOutput=False)
    yt_d = nc.declare_dram_parameter("yt", [D, T], bf16, isOutput=True)

    with tile.TileContext(nc) as tc, ExitStack() as top:
        const = top.enter_context(tc.tile_pool(name="const", bufs=1))
        tri_sb = const.tile([128, 128], bf16, name="tri_sb")
        nc.sync.dma_start(out=tri_sb, in_=tri_d[:, :])
        dum = const.tile([1, 2], f32, name="dum")
        ones_row = const.tile([1, 64], f32, name="ones_row")
        nc.vector.memset(ones_row, 1.0)
        scr = const.tile([128, 512], bf16, name="scr")
        nc.vector.memset(scr, 0.0)

        big = top.enter_context(tc.tile_pool(name="big", bufs=1))
        # [k within chunk, k chunk, head, e + ones col]
        vaug = big.tile([128, NK, 2 * NPAIR, 65], bf16, name="vaug")
        nc.vector.memset(vaug[:, :, :, 64:65], 1.0)

        # PSUM banks: S 2*2 + O 2*1 + M 2*1 = 8
        psS = top.enter_context(tc.tile_pool(name="psS", bufs=2, space="PSUM"))
        psO = top.enter_context(tc.tile_pool(name="psO", bufs=2, space="PSUM"))
        psM = top.enter_context(tc.tile_pool(name="psM", bufs=2, space="PSUM"))

        pw = top.enter_context(tc.tile_pool(name="pw", bufs=4))
        pwv = top.enter_context(tc.tile_pool(name="pwv", bufs=1))
        qkp = top.enter_context(tc.tile_pool(name="qkp", bufs=4))
        otn_p = top.enter_context(tc.tile_pool(name="otn_p", bufs=1))
        otn = otn_p.tile([128, NPAIR, T], bf16, name="otn")
        ptp = top.enter_context(tc.tile_pool(name="ptp", bufs=4))
        ocp = top.enter_context(tc.tile_pool(name="ocp", bufs=4))
        rcp = top.enter_context(tc.tile_pool(name="rcp", bufs=4))
        lbp = top.enter_context(tc.tile_pool(name="lbp", bufs=4))
        drp = top.enter_context(tc.tile_pool(name="drp", bufs=4, space="DRAM"))
        pwo = top.enter_context(tc.tile_pool(name="pwo", bufs=1))
        pyt = top.enter_context(tc.tile_pool(name="pyt", bufs=3))

        # HAM warmup: dependency-free matmuls on a zeroed scratch tile keep
        # the PE busy through the DMA lead-in so the clock gate opens to
        # 2.4 GHz before real work arrives (and never sees a >3us idle gap).
        wm = psM.tile([128, 512], f32, tag="mm", name="wm")
        for _ in range(40):
            nc.tensor.matmul(wm, scr[:, 0:128], scr, start=True, stop=True)

        def dma_w(wdram, p, kind):
            w_sb = pw.tile([128, ND, 128], bf16, tag="w", name=f"w_{kind}{p}")
            nc.sync.dma_start(out=w_sb, in_=wdram[p])
            return w_sb

        def attn_group(p, j, qt, kt, filler, pe_norm=False):
            """One (head-pair, q-chunk) attention group, heads row-tiled."""
            ncc = 4 * (j + 1)
            po = [psO.tile([65, 512], f32, tag="O", name=f"po{h}")
                  for h in range(2)]
            pts = {}

            def off_of(c):
                sub = c - 4 * j
                return sub * 128 if 0 <= sub < 4 else 0

            def emit_s(c):
                off = off_of(c)
                ps = psS.tile([128, 2, 512], f32, tag="S", name="ps")
                for h in range(2):
                    nc.tensor.matmul(
                        ps[:, h, off:],
                        kt[64 * h:64 * h + 64, c * 128:(c + 1) * 128],
                        qt[64 * h:64 * h + 64, j * 512 + off:(j + 1) * 512],
                        start=True,
                        stop=True,
                    )
                pt = ptp.tile([128, 2, 512], bf16, tag="pt", name="pt")
                nc.scalar.activation(out=pt[:, :, off:], in_=ps[:, :, off:],
                                     func=Exp, scale=0.125)
                sub = c - 4 * j
                if 0 <= sub < 4:
                    for h in range(2):
                        nc.vector.tensor_mul(
                            pt[:, h, sub * 128:(sub + 1) * 128],
                            pt[:, h, sub * 128:(sub + 1) * 128],
                            tri_sb,
                        )
                pts[c] = pt

            def emit_v(c):
                off = off_of(c)
                pt = pts.pop(c)
                for h in range(2):
                    nc.tensor.matmul(
                        po[h][:, off:],
                        vaug[:, c, 2 * p + h, :],
                        pt[:, h, off:],
                        start=(c == 0),
                        stop=(c == ncc - 1),
                    )

            emit_s(0)
            if ncc > 1:
                emit_s(1)
            for c in range(ncc):
                if c + 2 < ncc:
                    emit_s(c + 2)
                filler()
                emit_v(c)

            # normalize: otn[e, q] = O_T[e, q] / l[q]
            for h in range(2):
                oc = ocp.tile([64, 512], f32, tag="oc", name="oc")
                nc.vector.tensor_copy(out=oc, in_=po[h][0:64, :])
                rl = rcp.tile([1, 512], f32, tag="rl", name="rl")
                nc.vector.tensor_copy(out=rl, in_=po[h][64:65, :])
                if pe_norm:
                    # latency-critical tail: broadcast 1/l across partitions
                    # with a K=1 PE matmul instead of the DRAM round trip
                    nc.vector.reciprocal_approx_fast(rl, rl)
                    lbp_ps = psM.tile([64, 512], f32, tag="mm", name="lbp_ps")
                    nc.tensor.matmul(lbp_ps, ones_row, rl, start=True,
                                     stop=True)
                    nc.vector.tensor_mul(
                        otn[64 * h:64 * h + 64, p, j * 512:(j + 1) * 512],
                        oc, lbp_ps,
                    )
                else:
                    rd = drp.tile([1, 512], f32, tag="rd", name="rd")
                    nc.sync.dma_start(out=rd, in_=rl)
                    lb = lbp.tile([64, 512], f32, tag="lb", name="lb")
                    nc.sync.dma_start(
                        out=lb, in_=rd[0:1, :].partition_broadcast(64))
                    nc.vector.reciprocal_approx_fast(lb, lb)
                    nc.vector.tensor_mul(
                        otn[64 * h:64 * h + 64, p, j * 512:(j + 1) * 512],
                        oc, lb,
                    )

        with ExitStack() as mid:
            xtp = mid.enter_context(tc.tile_pool(name="xtp", bufs=1))
            xt = xtp.tile([128, NQ, ND, 512], bf16, name="xt")

            def proj_mms(ps_t4, w_sb, t4, dc_lo, dc_hi):
                for dc in range(dc_lo, dc_hi):
                    nc.tensor.matmul(
                        ps_t4,
                        w_sb[:, dc, :],
                        xt[:, t4, dc, :],
                        start=(dc == 0),
                        stop=(dc == ND - 1),
                    )

            # ---- Phase A DMAs ----------------------------------------------
            nc.sync.dma_start(out=xt[:, 0, :, :], in_=x_d[:, 0, :, :])
            wq0 = dma_w(wq_d, 0, "q")
            wk0 = dma_w(wk_d, 0, "k")
            wv_sb = pwv.tile([128, ND, 512], bf16, name="wv_sb")
            nc.sync.dma_start(out=wv_sb, in_=wv_d[:, :, :])
            for t4 in range(1, NQ):
                nc.sync.dma_start(out=xt[:, t4, :, :], in_=x_d[:, t4, :, :])
            # preload the exp table set while the PE streams projections
            nc.scalar.activation(out=dum, in_=tri_sb[0:1, 0:2], func=Exp)

            qt0 = qkp.tile([128, T], bf16, tag="qt", name="qt0")
            kt0 = qkp.tile([128, T], bf16, tag="kt", name="kt0")

            def mk_qk_units(w_sb, dest, t4):
                st = {}

                def a():
                    st["ps"] = psM.tile([128, 512], f32, tag="mm", name="psf")
                    proj_mms(st["ps"], w_sb, t4, 0, 4)

                def b():
                    proj_mms(st["ps"], w_sb, t4, 4, ND)
                    nc.vector.tensor_copy(
                        out=dest[:, t4 * 512:(t4 + 1) * 512], in_=st["ps"])
                return [a, b]

            def mk_v_units(t4, tr):
                st = {}
                tc_ = 4 * t4 + tr

                def vmms(dc_lo, dc_hi):
                    for dc in range(dc_lo, dc_hi):
                        nc.tensor.matmul(
                            st["ps"],
                            xt[:, t4, dc, tr * 128:(tr + 1) * 128],
                            wv_sb[:, dc, :],
                            start=(dc == 0),
                            stop=(dc == ND - 1),
                        )

                def a():
                    st["ps"] = psM.tile([128, 8, 64], f32, tag="mm",
                                        name="psv")
                    vmms(0, 4)

                def b():
                    vmms(4, ND)
                    nc.vector.tensor_copy(out=vaug[:, tc_, :, 0:64],
                                          in_=st["ps"])
                return [a, b]

            def t4_units(t4):
                us = mk_qk_units(wq0, qt0, t4) + mk_qk_units(wk0, kt0, t4)
                for tr in range(4):
                    us += mk_v_units(t4, tr)
                return us

            # t4=0 directly: pair-0 attention can start right after it
            for fn in t4_units(0):
                fn()

            # t4=1..3 and pair-1 Q/K run as filler inside pair-0's attention;
            # group (0, j) requires every unit tagged <= j emitted first.
            qt1 = qkp.tile([128, T], bf16, tag="qt", name="qt1")
            kt1 = qkp.tile([128, T], bf16, tag="kt", name="kt1")
            wq1 = dma_w(wq_d, 1, "q")
            wk1 = dma_w(wk_d, 1, "k")
            aq = []
            for t4 in range(1, NQ):
                for fn in t4_units(t4):
                    aq.append((t4, fn))
            for w_sb, dest in ((wq1, qt1), (wk1, kt1)):
                for t4 in range(NQ):
                    for fn in mk_qk_units(w_sb, dest, t4):
                        aq.append((9, fn))

            def drain(limit):
                i = 0
                while i < len(aq):
                    if aq[i][0] <= limit:
                        aq.pop(i)[1]()
                    else:
                        i += 1

            def filler0():
                if aq:
                    aq.pop(0)[1]()
                if len(aq) > 24:
                    aq.pop(0)[1]()

            for j in range(NQ):
                if j > 0:
                    drain(j)
                attn_group(0, j, qt0, kt0, filler0)
            while aq:
                aq.pop(0)[1]()

            # ---- Pairs 1-2: attention + next-pair Q/K filler ---------------
            qt_cur, kt_cur = qt1, kt1
            for p in (1, 2):
                fill = []
                qt_nxt = qkp.tile([128, T], bf16, tag="qt", name=f"qt{p+1}")
                kt_nxt = qkp.tile([128, T], bf16, tag="kt", name=f"kt{p+1}")
                wq_nxt = dma_w(wq_d, p + 1, "q")
                wk_nxt = dma_w(wk_d, p + 1, "k")
                for w_sb, dest in ((wq_nxt, qt_nxt), (wk_nxt, kt_nxt)):
                    for t4 in range(NQ):
                        fill += mk_qk_units(w_sb, dest, t4)

                def filler(fill=fill):
                    if fill:
                        fill.pop(0)()

                for j in range(NQ):
                    attn_group(p, j, qt_cur, kt_cur, filler)
                while fill:
                    fill.pop(0)()
                qt_cur, kt_cur = qt_nxt, kt_nxt

        # ---- Pair 3: attention + output projection as filler ----------------
        wo_sb = pwo.tile([128, NPAIR, ND, 128], bf16, name="wo_sb")
        nc.sync.dma_start(out=wo_sb, in_=wo_d[:, :, :, :])

        def out_unit(dc, qc):
            def emit():
                py = psM.tile([128, 512], f32, tag="mm", name="pyo")
                for pp in range(NPAIR):
                    nc.tensor.matmul(
                        py,
                        wo_sb[:, pp, dc, :],
                        otn[:, pp, qc * 512:(qc + 1) * 512],
                        start=(pp == 0),
                        stop=(pp == NPAIR - 1),
                    )
                yt_sb = pyt.tile([128, 512], bf16, tag="yt", name="yt_o")
                nc.vector.tensor_copy(out=yt_sb, in_=py)
                nc.sync.dma_start(
                    out=yt_d[dc * 128:(dc + 1) * 128,
                             qc * 512:(qc + 1) * 512],
                    in_=yt_sb,
                )
            return emit

        fillq = []

        def filler3():
            if fillq:
                fillq.pop(0)()

        for j in range(NQ):
            attn_group(3, j, qt_cur, kt_cur, filler3, pe_norm=True)
            for dc in range(ND):
                fillq.append(out_unit(dc, j))
        while fillq:
            fillq.pop(0)()

    nc.compile()
    return nc


def _pack_inputs(x, Wq, Wk, Wv, Wo):
    """Per-core input maps. Core c: batch c//2, head group c%2."""
    import ml_dtypes

    tri = np.triu(np.ones((128, 128), np.float32)).astype(ml_dtypes.bfloat16)

    def pack_w(W, g):
        # [NPAIR, 128(d_local), ND, 128(e2)]
        out = np.empty((NPAIR, 128, ND, 128), np.float32)
        for p in range(NPAIR):
            h1 = 8 * g + 2 * p
            r = W[[h1, h1 + 1]].transpose(1, 0, 2).reshape(D, 128)  # [d, e2]
            out[p] = r.reshape(ND, 128, 128).transpose(1, 0, 2)
        return np.ascontiguousarray(out).astype(ml_dtypes.bfloat16)

    def pack_wv(W, g):
        # [128(d_local), ND, 512(e8)] for the 8 heads of group g
        r = W[8 * g:8 * g + 8].transpose(1, 0, 2).reshape(D, 512)  # [d, e8]
        out = r.reshape(ND, 128, 512).transpose(1, 0, 2)
        return np.ascontiguousarray(out).astype(ml_dtypes.bfloat16)

    def pack_wo(Wo, g):
        # [128(e2), NPAIR, ND, 128(d)]
        out = np.empty((128, NPAIR, ND, 128), np.float32)
        for p in range(NPAIR):
            r0 = (8 * g + 2 * p) * 64
            out[:, p] = Wo[r0:r0 + 128].reshape(128, ND, 128)
        return np.ascontiguousarray(out).astype(ml_dtypes.bfloat16)

    packs = {}
    for g in range(2):
        packs[g] = dict(
            wq=pack_w(Wq, g), wk=pack_w(Wk, g), wv=pack_wv(Wv, g),
            wo=pack_wo(Wo, g),
        )
    in_maps = []
    for c in range(NCORES):
        b, g = c // 2, c % 2
        m = dict(packs[g])
        xt = x[b].reshape(NQ, 512, ND, 128).transpose(3, 0, 2, 1)
        m["x"] = np.ascontiguousarray(xt).astype(ml_dtypes.bfloat16)
        m["tri"] = tri
        in_maps.append(m)
    return in_maps


def kernel(x, Wq, Wk, Wv, Wo, bo):
    from concourse.bass_utils import run_bass_kernel_spmd

    x = np.asarray(x, np.float32)
    Wq, Wk, Wv = (np.asarray(a, np.float32) for a in (Wq, Wk, Wv))
    Wo = np.asarray(Wo, np.float32)
    bo = np.asarray(bo, np.float32)

    if "nc" not in _CACHE:
        _CACHE["nc"] = _build_program()
    nc = _CACHE["nc"]

    in_maps = _pack_inputs(x, Wq, Wk, Wv, Wo)
    res = run_bass_kernel_spmd(nc, in_maps, list(range(NCORES)))
    _CACHE["last_result"] = res

    out = np.empty((B, T, D), np.float32)
    for b in range(B):
        yt = (res.results[2 * b]["yt"].astype(np.float32)
              + res.results[2 * b + 1]["yt"].astype(np.float32))
        out[b] = yt.T + bo
    return out


# revision 13
# speedup vs baseline: 1.0607x; 1.0607x over previous
"""Multi-head causal attention (B=4, T=2048, D=1024, H=16, HS=64) on 8 TRN2
NeuronCores.

Sharding: batch (4-way) x head-group (2-way).  Core c handles batch c//2 and
heads 8*(c%2) .. 8*(c%2)+7.  Each core computes its 8 heads' attention and the
partial output projection Y_T = sum_h Wo_h^T @ O_T_h; the host sums the two
head-group partials per batch, transposes, and adds the output bias.

Per-core program (matmul datapath bf16, fp32 PSUM accumulation):
  - x^T arrives pre-transposed from host as [d, t] chunks.
  - V is computed directly in [t(=k), e] layout via matmul(lhsT=x^T chunk,
    rhs=Wv[d, e8]) -- no PE transposes at all.  A ones column per head turns
    the attn@v matmul into a fused (O^T, l) computation.
  - Q^T/K^T [e2, t] = matmul(lhsT=W[d, e2], rhs=x^T), head pairs packed on
    the PE M axis (head h of the pair in partitions 64h..64h+63).
  - S^T blocks [k=128, q<=512]: the two heads of a pair run CONCURRENTLY in
    the PE array via row tiling -- head 0 in rows 0-63 (tile_position (0,0)),
    head 1 in rows 64-127 ((64,0)), separate PSUM banks.  K=64 therefore
    costs ~N cycles per head PAIR instead of per head.
  - exp on ScalarE covers both heads in one ACTIVATE ([128, 2, 512-off] AP,
    1/sqrt(HS) folded into the activation scale); causal mask = upper-tri
    0/1 multiply on the diagonal sub-block only.
  - O^T_aug [65, q] accumulates per head over k chunks in PSUM; softmax
    normalization: reciprocal_approx_fast of the l row + DRAM-bounced
    partition broadcast + one DVE multiply into otn (bf16).
  - Output projection: per (dc, qc) a single 4-matmul PSUM chain over all
    pairs, bf16 copy, DMA out.  Emitted as PE filler inside pair-3's
    attention stream (lagging one q-chunk) and drained at the end.

Engine-level scheduling: emission order is the schedule.  S^T runs two
chunks ahead of attn@v; independent PE work (next pair's Q/K projections,
output-projection chains) is emitted as filler inside the attention stream
so the PE never idles and the HAM clock gate stays at 2.4 GHz.
"""

import numpy as np

B, T, D = 4, 2048, 1024
H, HS = 16, 64
NCORES = 8
NPAIR = 4   # head pairs per core
ND = 8      # 128-wide d chunks
NQ = 4      # 512-wide q chunks
NK = 16     # 128-wide k chunks

_CACHE = {}


def _build_program():
    import concourse.bass as bass
    import concourse.tile as tile
    from concourse import bacc, mybir
    from contextlib import ExitStack

    f32 = mybir.dt.float32
    bf16 = mybir.dt.bfloat16
    Exp = mybir.ActivationFunctionType.Exp

    nc = bacc.Bacc("TRN2", target_bir_lowering=False, debug=False)

    x_d = nc.declare_dram_parameter("x", [128, NQ, ND, 512], bf16, isOutput=False)
    wq_d = nc.declare_dram_parameter("wq", [NPAIR, 128, ND, 128], bf16, isOutput=False)
    wk_d = nc.declare_dram_parameter("wk", [NPAIR, 128, ND, 128], bf16, isOutput=False)
    wv_d = nc.declare_dram_parameter("wv", [128, ND, 512], bf16, isOutput=False)
    wo_d = nc.declare_dram_parameter("wo", [128, NPAIR, ND, 128], bf16, isOutput=False)
    tri_d = nc.declare_dram_parameter("tri", [128, 128], bf16, isOutput=False)
    yt_d = nc.declare_dram_parameter("yt", [D, T], bf16, isOutput=True)

    with tile.TileContext(nc) as tc, ExitStack() as top:
        const = top.enter_context(tc.tile_pool(name="const", bufs=1))
        tri_sb = const.tile([128, 128], bf16, name="tri_sb")
        nc.sync.dma_start(out=tri_sb, in_=tri_d[:, :])
        dum = const.tile([1, 2], f32, name="dum")
        ones_row = const.tile([1, 64], f32, name="ones_row")
        nc.vector.memset(ones_row, 1.0)
        scr = const.tile([128, 512], bf16, name="scr")
        nc.vector.memset(scr, 0.0)

        big = top.enter_context(tc.tile_pool(name="big", bufs=1))
        # [k within chunk, k chunk, head, e + ones col]
        vaug = big.tile([128, NK, 2 * NPAIR, 65], bf16, name="vaug")
        nc.vector.memset(vaug[:, :, :, 64:65], 1.0)

        # PSUM banks: S 2*2 + O 2*1 + M 2*1 = 8
        psS = top.enter_context(tc.tile_pool(name="psS", bufs=2, space="PSUM"))
        psO = top.enter_context(tc.tile_pool(name="psO", bufs=2, space="PSUM"))
        psM = top.enter_context(tc.tile_pool(name="psM", bufs=2, space="PSUM"))

        pw = top.enter_context(tc.tile_pool(name="pw", bufs=4))
        pwv = top.enter_context(tc.tile_pool(name="pwv", bufs=1))
        qkp = top.enter_context(tc.tile_pool(name="qkp", bufs=4))
        otn_p = top.enter_context(tc.tile_pool(name="otn_p", bufs=1))
        otn = otn_p.tile([128, NPAIR, T], bf16, name="otn")
        ptp = top.enter_context(tc.tile_pool(name="ptp", bufs=4))
        ocp = top.enter_context(tc.tile_pool(name="ocp", bufs=4))
        rcp = top.enter_context(tc.tile_pool(name="rcp", bufs=4))
        lbp = top.enter_context(tc.tile_pool(name="lbp", bufs=4))
        drp = top.enter_context(tc.tile_pool(name="drp", bufs=4, space="DRAM"))
        pwo = top.enter_context(tc.tile_pool(name="pwo", bufs=1))
        pyt = top.enter_context(tc.tile_pool(name="pyt", bufs=3))

        # HAM warmup: dependency-free matmuls on a zeroed scratch tile keep
        # the PE busy through the DMA lead-in so the clock gate opens to
        # 2.4 GHz before real work arrives (and never sees a >3us idle gap).
        wm = psM.tile([128, 512], f32, tag="mm", name="wm")
        for _ in range(12):
            nc.tensor.matmul(wm, scr[:, 0:128], scr, start=True, stop=True)

        def dma_w(wdram, p, kind):
            w_sb = pw.tile([128, ND, 128], bf16, tag="w", name=f"w_{kind}{p}")
            nc.sync.dma_start(out=w_sb, in_=wdram[p])
            return w_sb

        def attn_group(p, j, qt, kt, filler, pe_norm=False):
            """One (head-pair, q-chunk) attention group, heads row-tiled."""
            ncc = 4 * (j + 1)
            po = [psO.tile([65, 512], f32, tag="O", name=f"po{h}")
                  for h in range(2)]
            pts = {}

            def off_of(c):
                sub = c - 4 * j
                return sub * 128 if 0 <= sub < 4 else 0

            def emit_s(c):
                off = off_of(c)
                ps = psS.tile([128, 2, 512], f32, tag="S", name="ps")
                for h in range(2):
                    nc.tensor.matmul(
                        ps[:, h, off:],
                        kt[64 * h:64 * h + 64, c * 128:(c + 1) * 128],
                        qt[64 * h:64 * h + 64, j * 512 + off:(j + 1) * 512],
                        start=True,
                        stop=True,
                    )
                pt = ptp.tile([128, 2, 512], bf16, tag="pt", name="pt")
                nc.scalar.activation(out=pt[:, :, off:], in_=ps[:, :, off:],
                                     func=Exp, scale=0.125)
                sub = c - 4 * j
                if 0 <= sub < 4:
                    for h in range(2):
                        nc.vector.tensor_mul(
                            pt[:, h, sub * 128:(sub + 1) * 128],
                            pt[:, h, sub * 128:(sub + 1) * 128],
                            tri_sb,
                        )
                pts[c] = pt

            def emit_v(c):
                off = off_of(c)
                pt = pts.pop(c)
                for h in range(2):
                    nc.tensor.matmul(
                        po[h][:, off:],
                        vaug[:, c, 2 * p + h, :],
                        pt[:, h, off:],
                        start=(c == 0),
                        stop=(c == ncc - 1),
                    )

            emit_s(0)
            if ncc > 1:
                emit_s(1)
            for c in range(ncc):
                if c + 2 < ncc:
                    emit_s(c + 2)
                filler()
                emit_v(c)

            # normalize: otn[e, q] = O_T[e, q] / l[q]
            for h in range(2):
                oc = ocp.tile([64, 512], f32, tag="oc", name="oc")
                nc.vector.tensor_copy(out=oc, in_=po[h][0:64, :])
                rl = rcp.tile([1, 512], f32, tag="rl", name="rl")
                nc.vector.tensor_copy(out=rl, in_=po[h][64:65, :])
                if pe_norm:
                    # latency-critical tail: broadcast 1/l across partitions
                    # with a K=1 PE matmul instead of the DRAM round trip
                    nc.vector.reciprocal_approx_fast(rl, rl)
                    lbp_ps = psM.tile([64, 512], f32, tag="mm", name="lbp_ps")
                    nc.tensor.matmul(lbp_ps, ones_row, rl, start=True,
                                     stop=True)
                    nc.vector.tensor_mul(
                        otn[64 * h:64 * h + 64, p, j * 512:(j + 1) * 512],
                        oc, lbp_ps,
                    )
                else:
                    rd = drp.tile([1, 512], f32, tag="rd", name="rd")
                    nc.sync.dma_start(out=rd, in_=rl)
                    lb = lbp.tile([64, 512], f32, tag="lb", name="lb")
                    nc.sync.dma_start(
                        out=lb, in_=rd[0:1, :].partition_broadcast(64))
                    nc.vector.reciprocal_approx_fast(lb, lb)
                    nc.vector.tensor_mul(
                        otn[64 * h:64 * h + 64, p, j * 512:(j + 1) * 512],
                        oc, lb,
                    )

        with ExitStack() as mid:
            xtp = mid.enter_context(tc.tile_pool(name="xtp", bufs=1))
            xt = xtp.tile([128, NQ, ND, 512], bf16, name="xt")

            def proj_mms(ps_t4, w_sb, t4, dc_lo, dc_hi):
                for dc in range(dc_lo, dc_hi):
                    nc.tensor.matmul(
                        ps_t4,
                        w_sb[:, dc, :],
                        xt[:, t4, dc, :],
                        start=(dc == 0),
                        stop=(dc == ND - 1),
                    )

            # ---- Phase A DMAs ----------------------------------------------
            nc.sync.dma_start(out=xt[:, 0, :, :], in_=x_d[:, 0, :, :])
            wq0 = dma_w(wq_d, 0, "q")
            wk0 = dma_w(wk_d, 0, "k")
            wv_sb = pwv.tile([128, ND, 512], bf16, name="wv_sb")
            nc.sync.dma_start(out=wv_sb, in_=wv_d[:, :, :])
            for t4 in range(1, NQ):
                nc.sync.dma_start(out=xt[:, t4, :, :], in_=x_d[:, t4, :, :])
            # preload the exp table set while the PE streams projections
            nc.scalar.activation(out=dum, in_=tri_sb[0:1, 0:2], func=Exp)

            qt0 = qkp.tile([128, T], bf16, tag="qt", name="qt0")
            kt0 = qkp.tile([128, T], bf16, tag="kt", name="kt0")

            def mk_qk_units(w_sb, dest, t4):
                st = {}

                def a():
                    st["ps"] = psM.tile([128, 512], f32, tag="mm", name="psf")
                    proj_mms(st["ps"], w_sb, t4, 0, 4)

                def b():
                    proj_mms(st["ps"], w_sb, t4, 4, ND)
                    nc.vector.tensor_copy(
                        out=dest[:, t4 * 512:(t4 + 1) * 512], in_=st["ps"])
                return [a, b]

            def mk_v_units(t4, tr):
                st = {}
                tc_ = 4 * t4 + tr

                def vmms(dc_lo, dc_hi):
                    for dc in range(dc_lo, dc_hi):
                        nc.tensor.matmul(
                            st["ps"],
                            xt[:, t4, dc, tr * 128:(tr + 1) * 128],
                            wv_sb[:, dc, :],
                            start=(dc == 0),
                            stop=(dc == ND - 1),
                        )

                def a():
                    st["ps"] = psM.tile([128, 8, 64], f32, tag="mm",
                                        name="psv")
                    vmms(0, 4)

                def b():
                    vmms(4, ND)
                    nc.vector.tensor_copy(out=vaug[:, tc_, :, 0:64],
                                          in_=st["ps"])
                return [a, b]

            def t4_units(t4):
                us = mk_qk_units(wq0, qt0, t4) + mk_qk_units(wk0, kt0, t4)
                for tr in range(4):
                    us += mk_v_units(t4, tr)
                return us

            # t4=0 directly: pair-0 attention can start right after it
            for fn in t4_units(0):
                fn()

            # t4=1..3 and pair-1 Q/K run as filler inside pair-0's attention;
            # group (0, j) requires every unit tagged <= j emitted first.
            qt1 = qkp.tile([128, T], bf16, tag="qt", name="qt1")
            kt1 = qkp.tile([128, T], bf16, tag="kt", name="kt1")
            wq1 = dma_w(wq_d, 1, "q")
            wk1 = dma_w(wk_d, 1, "k")
            aq = []
            for t4 in range(1, NQ):
                for fn in t4_units(t4):
                    aq.append((t4, fn))
            for w_sb, dest in ((wq1, qt1), (wk1, kt1)):
                for t4 in range(NQ):
                    for fn in mk_qk_units(w_sb, dest, t4):
                        aq.append((9, fn))

            def drain(limit):
                i = 0
                while i < len(aq):
                    if aq[i][0] <= limit:
                        aq.pop(i)[1]()
                    else:
                        i += 1

            def filler0():
                if aq:
                    aq.pop(0)[1]()
                if len(aq) > 24:
                    aq.pop(0)[1]()

            for j in range(NQ):
                if j > 0:
                    drain(j)
                attn_group(0, j, qt0, kt0, filler0)
            while aq:
                aq.pop(0)[1]()

            # ---- Pairs 1-2: attention + next-pair Q/K filler ---------------
            qt_cur, kt_cur = qt1, kt1
            for p in (1, 2):
                fill = []
                qt_nxt = qkp.tile([128, T], bf16, tag="qt", name=f"qt{p+1}")
                kt_nxt = qkp.tile([128, T], bf16, tag="kt", name=f"kt{p+1}")
                wq_nxt = dma_w(wq_d, p + 1, "q")
                wk_nxt = dma_w(wk_d, p + 1, "k")
                for w_sb, dest in ((wq_nxt, qt_nxt), (wk_nxt, kt_nxt)):
                    for t4 in range(NQ):
                        fill += mk_qk_units(w_sb, dest, t4)

                def filler(fill=fill):
                    if fill:
                        fill.pop(0)()

                for j in range(NQ):
                    attn_group(p, j, qt_cur, kt_cur, filler)
                while fill:
                    fill.pop(0)()
                qt_cur, kt_cur = qt_nxt, kt_nxt

        # ---- Pair 3: attention + output projection as filler ----------------
        wo_sb = pwo.tile([128, NPAIR, ND, 128], bf16, name="wo_sb")
        nc.sync.dma_start(out=wo_sb, in_=wo_d[:, :, :, :])

        def out_unit(dc, qc):
            def emit():
                py = psM.tile([128, 512], f32, tag="mm", name="pyo")
                for pp in range(NPAIR):
                    nc.tensor.matmul(
                        py,
                        wo_sb[:, pp, dc, :],
                        otn[:, pp, qc * 512:(qc + 1) * 512],
                        start=(pp == 0),
                        stop=(pp == NPAIR - 1),
                    )
                yt_sb = pyt.tile([128, 512], bf16, tag="yt", name="yt_o")
                nc.vector.tensor_copy(out=yt_sb, in_=py)
                nc.sync.dma_start(
                    out=yt_d[dc * 128:(dc + 1) * 128,
                             qc * 512:(qc + 1) * 512],
                    in_=yt_sb,
                )
            return emit

        fillq = []

        def filler3():
            if fillq:
                fillq.pop(0)()

        for j in range(NQ):
            attn_group(3, j, qt_cur, kt_cur, filler3, pe_norm=(j == NQ - 1))
            for dc in range(ND):
                fillq.append(out_unit(dc, j))
        while fillq:
            fillq.pop(0)()

    nc.compile()
    return nc


def _pack_inputs(x, Wq, Wk, Wv, Wo):
    """Per-core input maps. Core c: batch c//2, head group c%2."""
    import ml_dtypes

    tri = np.triu(np.ones((128, 128), np.float32)).astype(ml_dtypes.bfloat16)

    def pack_w(W, g):
        # [NPAIR, 128(d_local), ND, 128(e2)]
        out = np.empty((NPAIR, 128, ND, 128), np.float32)
        for p in range(NPAIR):
            h1 = 8 * g + 2 * p
            r = W[[h1, h1 + 1]].transpose(1, 0, 2).reshape(D, 128)  # [d, e2]
            out[p] = r.reshape(ND, 128, 128).transpose(1, 0, 2)
        return np.ascontiguousarray(out).astype(ml_dtypes.bfloat16)

    def pack_wv(W, g):
        # [128(d_local), ND, 512(e8)] for the 8 heads of group g
        r = W[8 * g:8 * g + 8].transpose(1, 0, 2).reshape(D, 512)  # [d, e8]
        out = r.reshape(ND, 128, 512).transpose(1, 0, 2)
        return np.ascontiguousarray(out).astype(ml_dtypes.bfloat16)

    def pack_wo(Wo, g):
        # [128(e2), NPAIR, ND, 128(d)]
        out = np.empty((128, NPAIR, ND, 128), np.float32)
        for p in range(NPAIR):
            r0 = (8 * g + 2 * p) * 64
            out[:, p] = Wo[r0:r0 + 128].reshape(128, ND, 128)
        return np.ascontiguousarray(out).astype(ml_dtypes.bfloat16)

    packs = {}
    for g in range(2):
        packs[g] = dict(
            wq=pack_w(Wq, g), wk=pack_w(Wk, g), wv=pack_wv(Wv, g),
            wo=pack_wo(Wo, g),
        )
    in_maps = []
    for c in range(NCORES):
        b, g = c // 2, c % 2
        m = dict(packs[g])
        xt = x[b].reshape(NQ, 512, ND, 128).transpose(3, 0, 2, 1)
        m["x"] = np.ascontiguousarray(xt).astype(ml_dtypes.bfloat16)
        m["tri"] = tri
        in_maps.append(m)
    return in_maps


def kernel(x, Wq, Wk, Wv, Wo, bo):
    from concourse.bass_utils import run_bass_kernel_spmd

    x = np.asarray(x, np.float32)
    Wq, Wk, Wv = (np.asarray(a, np.float32) for a in (Wq, Wk, Wv))
    Wo = np.asarray(Wo, np.float32)
    bo = np.asarray(bo, np.float32)

    if "nc" not in _CACHE:
        _CACHE["nc"] = _build_program()
    nc = _CACHE["nc"]

    in_maps = _pack_inputs(x, Wq, Wk, Wv, Wo)
    res = run_bass_kernel_spmd(nc, in_maps, list(range(NCORES)))
    _CACHE["last_result"] = res

    out = np.empty((B, T, D), np.float32)
    for b in range(B):
        yt = (res.results[2 * b]["yt"].astype(np.float32)
              + res.results[2 * b + 1]["yt"].astype(np.float32))
        out[b] = yt.T + bo
    return out
